# revision 24
# baseline (speedup 1.0000x reference)
"""Nearest-neighbor classifier kernel for 8 TRN2 NeuronCores.

Computes: scores = x @ means.T; out = one_hot(argmax(scores, axis=1), 1000).

Strategy (data-parallel, per sharding hint):
  - shard x row-wise across 8 cores (2048 samples each), replicate means
  - host-side staging: block-tile shards so every DMA source region is
    contiguous per SBUF partition (maximal descriptors; DIRECT2D
    descriptor-gen on the sync sequencer stays ~0.6us/DMA), and pre-round
    operands to the FP22 grid (round to nearest, 11 explicit mantissa bits)
    so the TensorEngine's fp32r input truncation is exact (fp32r streams at
    full PE rate for N>=256, 4x the plain-fp32 matmul rate)
  - per core: 16 sample-tiles of 128; scores accumulate over 16 k-chunks into
    two PSUM banks of 500 classes (a matmul may not cross a 2KB PSUM bank);
    group 0 is k-outer chunk-chasing the DMA stream, groups 1-3 are m-outer
    with the x slab prefetched one group ahead in 1MB pieces so compute
    chases the stream across group boundaries
  - epilogue per tile: DVE reduce_max straight off PSUM in parallel with a
    Scalar-engine drain of the banks to contiguous SBUF (PSUM frees ~1.2us
    after the tile's last matmul), then broadcast + DVE max_index -> the
    argmax class id per sample, staged into a [128,16] uint32 tile; ONE 8KB
    output DMA per core instead of the 8.2MB one-hot (the one-hot is
    materialized host-side, which is exact)

Optionally runs extra compensation passes (hi/lo operand splits) for
fp32-exact scores; PASS_MODE=1 measured 2 argmax flips vs the fp32 reference
on the fixed inputs (rel err ~0.016), PASS_MODE=3 measured 0.
"""

import sys

if "/opt/trn_rl_repo" not in sys.path:
    sys.path.insert(0, "/opt/trn_rl_repo")

import numpy as np

import concourse.bass as bass
import concourse.mybir as mybir
from concourse import bacc
from concourse.tile import TileContext
from concourse.bass_utils import run_bass_kernel_spmd

N_CORES = 8
NS_TOTAL = 16384
ND = 2048
NCLS = 1000

# (x_part, m_part) operand pairs accumulated into the same PSUM scores.
# 1-pass: [(0, 0)] with RTN22 pre-rounding.
# 3-pass (fp32-exact): [(0, 0), (1, 0), (0, 1)] with x=(hi,lo), m=(hi,lo).
PASS_MODE = 1

SPLIT_FIRST_CHUNKS = True
P = 128            # SBUF partitions / PE contraction tile
GROUP = 512        # samples per x DMA slab
CLS_SPLITS = ((0, 500), (500, 1000))  # PSUM-bank-sized class column ranges


def _rtn22(a: np.ndarray) -> np.ndarray:
    """Round fp32 to nearest point on the FP22 (11 explicit mantissa bit)
    grid, so the PE's fp32r truncation of the result is the identity."""
    u = a.view(np.uint32)
    u = (u + np.uint32(0x800)) & np.uint32(0xFFFFF000)
    return u.view(np.float32)


def _trunc22(a: np.ndarray) -> np.ndarray:
    return (a.view(np.uint32) & np.uint32(0xFFFFF000)).view(np.float32)


def build_bass(ns: int, nd: int, ncls: int, n_x: int, n_m: int, pairs):
    """One-core SPMD program: xt{i} [nd, ns], mt{j} [nd, ncls] -> idx [P, ntiles]."""
    fr = mybir.dt.float32r
    f32 = mybir.dt.float32
    u32 = mybir.dt.uint32
    kc = nd // P
    # SBUF budget: resident means (n_m*kc*4KB/partition) + triple-buffered x
    # slabs (n_x*3*kc*GROUP*4B) + score-copy pool must fit in ~190KB/partition
    ntiles = ns // P

    # Bacc (not raw Bass): its compile() legalizes multi-wait instructions
    # (move_matmul_waits_to_ldweights, event semaphores), which walrus
    # codegen's 1-wait-per-instruction limit requires.
    nc = bacc.Bacc("TRN2", target_bir_lowering=False, debug=False)
    group = {1: GROUP, 2: 256, 3: 128}[len(pairs)]
    n_groups = ns // group
    # block-tiled DRAM layouts (host pre-staged): each 32KB-per-partition
    # slab / 4KB-per-partition means chunk is CONTIGUOUS per partition, so
    # DMA descriptors are maximal and DIRECT2D descriptor-gen on the sync
    # sequencer drops ~10x (a [128,16,512]-strided slab cost ~7us to enqueue)
    xts = [nc.dram_tensor(f"xt{i}", [n_groups * P, kc * group], fr,
                          kind="ExternalInput")
           for i in range(n_x)]
    mts = [nc.dram_tensor(f"mt{j}", [P, kc * ncls], fr, kind="ExternalInput")
           for j in range(n_m)]
    mpg = group // P  # sample tiles per group
    # out[p, t] = 999 - argmax class id of sample t*P + p (exact f32 int)
    out = nc.dram_tensor("out", [P, ntiles], f32, kind="ExternalOutput")

    with TileContext(nc) as tc:
        with (
            tc.tile_pool(name="means", bufs=1) as mpool,
            tc.tile_pool(name="xslab", bufs=3) as xpool,
            tc.tile_pool(name="scopy", bufs=4) as cpool,
            tc.tile_pool(name="stats", bufs=4) as spool,
            tc.tile_pool(name="scores", bufs=4, space="PSUM") as pspool,
        ):
            # k-outer ordering: for each k-chunk, DMA its means chunk (group 0
            # only) + x chunk, then run all in-flight sample-tiles' matmuls on
            # it. Compute starts after the first ~0.4MB instead of the full
            # 12MB preamble, and each chunk's 8 matmuls (~2.2us) cover its DMA
            # (~2.1us), so the PE pipeline fills almost immediately.
            n_steps = len(pairs) * kc
            split_w = CLS_SPLITS[0][1] - CLS_SPLITS[0][0]

            # all means chunks live in one resident slab tile per m-part
            # (single pool slot each -> fewer semaphores to ritually await at
            # the end-of-program barrier); DMAs stay per-chunk so group 0's
            # matmuls can chase the stream
            m_slabs = {
                j: mpool.tile([P, kc * ncls], fr, name=f"ms{j}", tag=f"ms{j}")
                for j in range(n_m)
            }
            m_loaded = set()

            def m_chunk(j, k):
                return m_slabs[j][:, k * ncls:(k + 1) * ncls]

            def load_m_chunk(j, k, lo, hi):
                nc.sync.dma_start(
                    out=m_slabs[j][:, k * ncls + lo:k * ncls + hi],
                    in_=mts[j][:, k * ncls + lo:k * ncls + hi],
                )

            # all 16 tiles' (999 - argmax) values accumulate here as exact
            # small-integer f32; single out-DMA at end (one-time tiles live
            # in the bufs=1 means pool - a pool charges every tag x bufs)
            idx_pack = mpool.tile([P, ntiles], f32, name="idxpack",
                                  tag="idxpack")
            # revio[p, c] = 999 - c: argmax = 999 - max(mask * revio), and
            # exact score ties resolve to the FIRST (lowest) class id like
            # np.argmax. Plain ALU ops only — MAX_INDEX lowers to a
            # MATCH_VALUE_LOAD + FIND_INDEX8 pair sharing a hidden DVE match
            # register, which concurrent epilogues can clobber mid-pair.
            # f32 iota of 0..999 is exact.
            revio = mpool.tile([P, ncls], f32, name="revio", tag="revio")
            nc.gpsimd.iota(revio, pattern=[[-1, ncls]], base=ncls - 1,
                           channel_multiplier=0,
                           allow_small_or_imprecise_dtypes=True)
            # dummy copy: forces the Scalar engine's ACT_TABLE_LOAD (~1.5us)
            # into the preamble instead of the first epilogue's critical path
            zz = mpool.tile([P, 8], f32, name="zz", tag="zz")
            nc.scalar.copy(zz, revio[:, 0:8])

            def emit_epilogue(g, mi, ps):
                t = g * mpg + mi
                ps3 = ps.rearrange("p (b c) -> p b c", c=512)[:, :, :split_w]
                # row max on DVE straight off PSUM, in parallel with the
                # Scalar-engine drain of the banks to contiguous SBUF
                # (GPSIMD cannot access PSUM); PSUM frees after ~1.2us
                # the PSUM-releasing pair runs at high priority so the
                # scheduler orders it ahead of older epilogues' DVE scans
                rmax = spool.tile([P, 1], f32, name="rmax", tag="rmax")
                sc = cpool.tile([P, ncls], f32, name="sc", tag="sc")
                sc3 = sc.rearrange("p (b c) -> p b c", c=split_w)
                with tc.high_priority():
                    nc.vector.reduce_max(rmax, ps3,
                                         axis=mybir.AxisListType.XY)
                    nc.scalar.copy(sc3, ps3)
                # mask of row maxima, * revio, reduce -> 999-argmax, all on
                # DVE (GpSimd tensor ops on [128,1000] measured ~10x slower).
                # Group 0's tails are deprioritized: the fill phase is
                # DMA-bound and these scans otherwise steal DVE time from
                # the PSUM-releasing reduce_max ops at the group boundary;
                # deferred, they absorb into group 1's DVE idle time.
                import contextlib
                defer = (tc.high_priority(offset=-5000) if g == 0
                         else contextlib.nullcontext())
                with defer:
                    msk = cpool.tile([P, ncls], f32, name="msk", tag="msk")
                    nc.vector.tensor_scalar(
                        msk, sc, rmax, None, mybir.AluOpType.is_equal,
                    )
                    nc.vector.tensor_tensor(msk, msk, revio,
                                            mybir.AluOpType.mult)
                    iv = spool.tile([P, 1], f32, name="iv", tag="iv")
                    nc.vector.reduce_max(iv, msk, axis=mybir.AxisListType.X)
                    nc.gpsimd.tensor_copy(idx_pack[:, t:t + 1], iv)

            def mm(ps, xs, mi, j, k, step):
                lhsT = xs[:, k * group + mi * P:k * group + (mi + 1) * P]
                mk = m_chunk(j, k)
                for si, (lo, hi) in enumerate(CLS_SPLITS):
                    # class split si at column si*512: a matmul must stay
                    # within one 2KB PSUM bank
                    nc.tensor.matmul(
                        ps[:, si * 512:si * 512 + (hi - lo)],
                        lhsT,
                        mk[:, lo:hi],
                        start=(step == 0),
                        stop=(step == n_steps - 1),
                    )

            # one x slab tile per (x-part, group): [128, kc*group]; group 0
            # loads it in per-chunk pieces so matmuls can chase the stream,
            # later groups load it with ONE strided DMA (fewer descriptors
            # to enqueue and fewer end-of-program semaphore waits)
            x_slabs = {}

            def get_slab(i, g):
                if (i, g) in x_slabs:
                    return x_slabs[(i, g)]
                xs = xpool.tile([P, kc * group], fr, name=f"xs{i}",
                                tag=f"xs{i}")
                x_slabs[(i, g)] = xs
                if g > 0:
                    # 4 pieces of 4 k-chunks (1MB each): group g+1's first
                    # matmuls only wait on piece 0, so compute chases the
                    # stream across the group boundary instead of stalling
                    # on a monolithic 4MB transfer
                    pw = kc * group // 4
                    for c in range(4):
                        nc.sync.dma_start(
                            out=xs[:, c * pw:(c + 1) * pw],
                            in_=xts[i][g * P:(g + 1) * P,
                                       c * pw:(c + 1) * pw],
                        )
                return xs

            def load_x_piece(i, g, k, n_split=1):
                xs = get_slab(i, g)
                cw = group // n_split
                for c in range(n_split):
                    nc.sync.dma_start(
                        out=xs[:, k * group + c * cw:
                               k * group + (c + 1) * cw],
                        in_=xts[i][g * P:(g + 1) * P,
                                   k * group + c * cw:
                                   k * group + (c + 1) * cw],
                    )

            for g in range(n_groups):
                pss = [
                    pspool.tile([P, 1024], f32, name=f"ps{mi}", tag="ps")
                    for mi in range(mpg)
                ]

                if g == 0:
                    # fill phase, k-outer: matmuls chase the DMA stream chunk
                    # by chunk; compute starts after the first ~0.4MB instead
                    # of the full 12MB preamble. The first chunks' DMAs are
                    # split column-wise so the first matmul's deps (means
                    # bank 0 + first sample-tile's x columns) arrive ahead of
                    # the bulk stream.
                    step = 0
                    x_loaded = set()
                    for (i, j) in pairs:
                        for k in range(kc):
                            split = k < 2 and SPLIT_FIRST_CHUNKS
                            if split and (j, k) not in m_loaded:
                                m_loaded.add((j, k))
                                # k==0: means bank0 first (first matmul's
                                # long-pole dep), then x pieces, then bank1
                                load_m_chunk(j, k, 0, split_w)
                            if split and (i, k) not in x_loaded:
                                load_x_piece(i, 0, k, n_split=2)
                                x_loaded.add((i, k))
                            if split:
                                load_m_chunk(j, k, split_w, ncls)
                            if (j, k) not in m_loaded:
                                m_loaded.add((j, k))
                                load_m_chunk(j, k, 0, ncls)
                            if (i, k) not in x_loaded:
                                load_x_piece(i, 0, k)
                                x_loaded.add((i, k))
                            for mi in range(mpg):
                                mm(pss[mi], x_slabs[(i, 0)], mi, j, k, step)
                            step += 1
                    for mi in range(mpg):
                        emit_epilogue(g, mi, pss[mi])
                else:
                    # steady state, m-outer: the x slab was prefetched during
                    # the previous group, each m-tile's epilogue overlaps the
                    # next m-tile's matmuls, and only the last epilogue trails
                    # prefetch the NEXT group's slab first so its 4MB DMA
                    # overlaps this group's compute (bufs=3 keeps its slot
                    # free of dependencies on the just-finished group)
                    if g + 1 < n_groups:
                        for (i, j) in pairs:
                            get_slab(i, g + 1)
                    for mi in range(mpg):
                        step = 0
                        for (i, j) in pairs:
                            for k in range(kc):
                                mm(pss[mi], x_slabs[(i, g)], mi, j, k, step)
                                step += 1
                        emit_epilogue(g, mi, pss[mi])
                if g == 0 and n_groups > 1:
                    for (i, j) in pairs:
                        get_slab(i, 1)

            # single tiny out-DMA; waits on all 16 pack writes
            nc.sync.dma_start(out=out[:, :], in_=idx_pack)

    nc.compile()
    return nc


def _stage_host(x: np.ndarray, means: np.ndarray, pass_mode: int):
    """Returns (x_parts, m_parts, pairs); x_parts entries are [NS_TOTAL, ND]."""
    if pass_mode == 1:
        return [_rtn22(x)], [_rtn22(means)], [(0, 0)]
    if pass_mode == 2:
        xh = _trunc22(x)
        return [xh, x - xh], [_rtn22(means)], [(0, 0), (1, 0)]
    if pass_mode == 3:
        xh = _trunc22(x)
        mh = _trunc22(means)
        return [xh, x - xh], [mh, means - mh], [(0, 0), (1, 0), (0, 1)]
    raise ValueError(f"bad pass_mode {pass_mode}")


def run(x, means, pass_mode=PASS_MODE, trace=False, **spmd_kwargs):
    x = np.ascontiguousarray(np.asarray(x, dtype=np.float32))
    means = np.ascontiguousarray(np.asarray(means, dtype=np.float32))
    assert x.shape == (NS_TOTAL, ND) and means.shape == (NCLS, ND)

    x_parts, m_parts, pairs = _stage_host(x, means, pass_mode)

    ns = NS_TOTAL // N_CORES
    group = {1: GROUP, 2: 256, 3: 128}[len(pairs)]
    n_groups = ns // group
    kc = ND // P
    # block-tiled DRAM staging (see build_bass): per core,
    # xtb[g*P + p, k*group + c] = x[c*ns + g*group + c_, k*P + p]
    # mtb[p, k*ncls + cls]      = means[cls, k*P + p]
    m_parts_b = [
        np.ascontiguousarray(
            m.T.reshape(kc, P, NCLS).transpose(1, 0, 2).reshape(P, kc * NCLS)
        )
        for m in m_parts
    ]
    in_maps = []
    for c in range(N_CORES):
        im = {}
        for i, xp in enumerate(x_parts):
            xc = xp[c * ns:(c + 1) * ns, :]          # [ns, nd] sample-major
            xb = (xc.reshape(n_groups, group, kc, P)  # [g, c, k, p]
                  .transpose(0, 3, 2, 1)              # [g, p, k, c]
                  .reshape(n_groups * P, kc * group))
            im[f"xt{i}"] = np.ascontiguousarray(xb)
        for j, mp in enumerate(m_parts_b):
            im[f"mt{j}"] = mp
        in_maps.append(im)

    nc = build_bass(ns, ND, NCLS, len(x_parts), len(m_parts), pairs)
    res = run_bass_kernel_spmd(
        nc, in_maps, core_ids=list(range(N_CORES)), trace=trace, **spmd_kwargs
    )
    # device returns (999 - argmax)[p, t] for sample t*128+p per core as
    # exact small-integer f32; build the one-hot host-side (exact)
    full = np.empty((NS_TOTAL, NCLS), dtype=np.float32)
    for c, r in enumerate(res.results):
        iv = np.asarray(r["out"])                     # [P, ntiles] f32
        cls = (NCLS - 1) - np.rint(iv).astype(np.int64)
        cls = cls.T.reshape(-1)                       # [ns] sample-major
        oh = np.zeros((ns, NCLS), dtype=np.float32)
        oh[np.arange(ns), cls] = 1.0
        full[c * ns:(c + 1) * ns] = oh
    return full, res


def kernel(x=None, means=None, n_classes=None, **_ignored) -> np.ndarray:
    assert n_classes is None or int(n_classes) == NCLS
    out, _ = run(x, means)
    return out
